# revision 5
# baseline (speedup 1.0000x reference)
"""CenterLoss kernel for Trainium2 (8 NeuronCores, data-parallel over N).

loss = sum_{n,c,w} act[n,c,w] * dist[n,c,w],  clipped at 1e-6, where
  dist[n,c,w] = ||x[n,:,w] - ctr[:,c]||^2 = x2[n,w] - 2*xc[n,c,w] + c2[c]

v6 strategy ("layout B"): make w the matmul CONTRACTION dim so the whole
loss collapses into one tiny accumulated matrix, eliminating the full-size
DVE pass (v5's pacer at ~34us) and the 2.25x PE passes:

  M[j, c] = sum_w xs[w, j] * act[w, c]        (w = 32768 per core)
  with xs[w, :] = [x(64 dims) | x2[w] | 1]  (66 stationary cols)
  loss_core = sum_{j,c} G[j, c] * M[j, c],
  G = [[-2*ctr(64x80)], [ones(80)], [c2(80)]]  (66 x 80, host-built)

Per core: 256 chunks of 128 w's; each chunk is ONE fp8 matmul
(stationary xs [128,66], moving act [128,80]) accumulating into a single
[66,80] fp32 PSUM bank (start on k=0, stop on k=255). PE cost/chunk =
max(LDWEIGHTS 128 rows, 80 moving cols) ~ 128 cyc -> 32768 cyc total,
which keeps pace with DMA even at the 1.2 GHz pstate. Tail: one DVE
scalar_tensor_tensor (psum*G, row-accum), GpSimd partition_all_reduce,
1-elem DMA out. Host sums the 8 per-core partials and applies the clip.

HBM per core ~4.8 MB (act 2.62 + xs 2.16, both fp8, host-transposed so
DMA lands in SBUF layout directly) -> ~14.5us at ~330 GB/s, the roofline.
fp8 rounding errors are unbiased and average out over the 2.6M-term
accumulation (v5 measured ~3e-4 with the same quantization).
"""

import os
import sys

import numpy as np

for _p in ("/opt/trn_rl_repo",):
    if _p not in sys.path and os.path.isdir(_p):
        sys.path.insert(0, _p)

N, D, C, W = 16, 64, 80, 16384
NCORES = 8
NPER = N // NCORES  # 2
WG = NPER * W  # 32768 w-positions per core
CHUNK = 128
NCH = WG // CHUNK  # 256 chunks
SC = D + 2  # 66 stationary cols: [x(64) | x2 | 1]
MC = C  # 80 moving cols
NPIECE = 16  # DMA pieces per stream
CPP = NCH // NPIECE  # 16 chunks per piece
NWARM = 4  # pstate warm-up dummy matmuls

_CACHE = {}


def _build_bass():
    import concourse.bacc as bacc
    import concourse.tile as tile
    from concourse import bass_isa, mybir

    fp32 = mybir.dt.float32
    fp8 = mybir.dt.float8e4
    Alu = mybir.AluOpType

    nc = bacc.Bacc("TRN2", target_bir_lowering=False)

    att = nc.dram_tensor("att", [128, NCH * MC], fp8, kind="ExternalInput")
    xst = nc.dram_tensor("xst", [128, NCH * SC], fp8, kind="ExternalInput")
    gt = nc.dram_tensor("gt", [128, MC], fp32, kind="ExternalInput")
    out = nc.dram_tensor("out", [1, 1], fp32, kind="ExternalOutput")

    from contextlib import ExitStack

    with tile.TileContext(nc) as tc, ExitStack() as ctx:
        static = ctx.enter_context(tc.tile_pool(name="static", bufs=1))
        pacc = ctx.enter_context(tc.tile_pool(name="pacc", bufs=1, space="PSUM"))
        pdum = ctx.enter_context(tc.tile_pool(name="pdum", bufs=1, space="PSUM"))

        act_t = static.tile([128, NCH * MC], fp8)
        xs_t = static.tile([128, NCH * SC], fp8)
        g_t = static.tile([128, MC], fp32)
        wsc = static.tile([128, 512], fp8)  # warm-up scratch, memset once
        tt = static.tile([128, MC], fp32)
        trow = static.tile([128, 1], fp32)
        rall = static.tile([128, 1], fp32)

        nc.vector.memset(wsc[:, :], 0.0)
        # rows 66:128 of trow must be zero for the final all-reduce.
        nc.vector.memset(trow[:, :], 0.0)

        # ---- all data DMAs issued upfront (static tiles => no deps).
        # act/xs pieces alternate in chunk-consumption order, greedy
        # byte-balanced across three HWDGE rings (sync + scalar + gpsimd;
        # gpsimd's ring is free until the tail all-reduce).
        ring_bytes = [0, 0, 0]
        rings = [nc.sync, nc.scalar, nc.gpsimd]

        def pick_ring():
            return min(range(len(rings)), key=lambda i: ring_bytes[i])

        for p in range(NPIECE):
            a0, a1 = p * CPP * MC, (p + 1) * CPP * MC
            i = pick_ring()
            rings[i].dma_start(out=act_t[:, a0:a1], in_=att[:, a0:a1])
            ring_bytes[i] += 128 * (a1 - a0)
            s0, s1 = p * CPP * SC, (p + 1) * CPP * SC
            i = pick_ring()
            rings[i].dma_start(out=xs_t[:, s0:s1], in_=xst[:, s0:s1])
            ring_bytes[i] += 128 * (s1 - s0)
        rings[pick_ring()].dma_start(out=g_t[:, :], in_=gt[:, :])

        # ---- PE warm-up: a few dummy matmuls (no DMA deps) bridge the
        # preamble->data gap and start the pstate ramp.
        pd_w = pdum.tile([128, 512], fp32, tag="pdw")
        for _ in range(NWARM):
            nc.tensor.matmul(
                pd_w[0:64, 0:512], wsc[:, 0:64], wsc[:, 0:512],
                start=True, stop=True,
            )

        # ---- the accumulation: 256 chunk-matmuls into one psum bank.
        pm = pacc.tile([128, MC], fp32, tag="pm")
        for k in range(NCH):
            nc.tensor.matmul(
                pm[0:SC, 0:MC],
                xs_t[:, k * SC : (k + 1) * SC],
                act_t[:, k * MC : (k + 1) * MC],
                start=(k == 0),
                stop=(k == NCH - 1),
            )

        # ---- tail: loss_core = sum(G * M)
        nc.vector.scalar_tensor_tensor(
            out=tt[0:SC, :],
            in0=pm[0:SC, 0:MC],
            scalar=0.0,
            in1=g_t[0:SC, :],
            op0=Alu.add,
            op1=Alu.mult,
            accum_out=trow[0:SC, 0:1],
        )
        nc.gpsimd.partition_all_reduce(
            rall[:], trow[:], channels=128, reduce_op=bass_isa.ReduceOp.add
        )
        nc.sync.dma_start(out=out[:, :], in_=rall[0:1, :])

    nc.compile()
    return nc


def _get_nc():
    if "nc" not in _CACHE:
        _CACHE["nc"] = _build_bass()
    return _CACHE["nc"]


def build_in_maps(x, c, act):
    import ml_dtypes

    fp8 = ml_dtypes.float8_e4m3
    x = np.ascontiguousarray(np.asarray(x), dtype=np.float32)
    c = np.ascontiguousarray(np.asarray(c), dtype=np.float32)
    act = np.ascontiguousarray(np.asarray(act), dtype=np.float32)
    assert x.shape == (N, D, W) and c.shape == (D, C) and act.shape == (N, C, W)

    c2 = np.sum(c * c, axis=0, dtype=np.float32)  # [C]
    g = np.zeros((128, MC), dtype=np.float32)
    g[0:D] = -2.0 * c
    g[D] = 1.0
    g[D + 1] = c2

    in_maps = []
    for kc in range(NCORES):
        xk = x[NPER * kc : NPER * (kc + 1)]  # [2, 64, W] fp32
        ak = act[NPER * kc : NPER * (kc + 1)]  # [2, 80, W] fp32

        # w-global major: [WG, cols], then chunk-fold to [128, NCH*cols]
        xw = xk.transpose(0, 2, 1).reshape(WG, D)  # [32768, 64]
        x2 = np.sum(xw * xw, axis=1, dtype=np.float32)  # [32768]
        xs = np.empty((WG, SC), dtype=fp8)
        xs[:, 0:D] = xw.astype(fp8)
        xs[:, D] = x2.astype(fp8)
        xs[:, D + 1] = np.float32(1.0)
        xs_tiled = np.ascontiguousarray(
            xs.reshape(NCH, CHUNK, SC).transpose(1, 0, 2).reshape(CHUNK, NCH * SC)
        )

        aw = ak.transpose(0, 2, 1).reshape(WG, C).astype(fp8)  # [32768, 80]
        at_tiled = np.ascontiguousarray(
            aw.reshape(NCH, CHUNK, MC).transpose(1, 0, 2).reshape(CHUNK, NCH * MC)
        )

        in_maps.append({"att": at_tiled, "xst": xs_tiled, "gt": g})
    return in_maps


def kernel(x, c, act):
    from concourse.bass_utils import run_bass_kernel_spmd

    in_maps = build_in_maps(x, c, act)
    res = run_bass_kernel_spmd(_get_nc(), in_maps, core_ids=list(range(NCORES)))
    total = np.float32(0.0)
    for r in res.results:
        total = np.float32(total + np.float32(r["out"][0, 0]))
    return np.maximum(np.float32(total), np.float32(1e-6))
